# revision 38
# baseline (speedup 1.0000x reference)
"""Trainium2 Bass kernel for out = x @ W.T + b  (x:[8192,1024] f32, W:[1024,1024] f32, b:[1024] f32).

Data-parallel over batch across 8 NeuronCores: each core computes a
[1024,1024] @ [1024,1024]^T matmul + bias for its 1024-row batch shard.

Host-side prep (inside kernel(), not on device): shard x along batch,
pre-transpose x and W so the contraction dim (in_f) lands on SBUF
partitions with fully-contiguous per-partition DMA reads, and cast to
fp16 (rel err ~2.5e-4 against the 2e-2 gate).

Schedule (per core), tuned on real-HW NTFF traces so the PE's matmul
stream (65536 PE cycles = 27.3 us @ 2.4 GHz — the compute roofline)
runs gap-free behind a ~6.5 us NEFF preamble:

  - Each DRAM "xw" row packs the W half-0 chunk AND the x slab that one
    8-matmul group consumes, so a single DMA feeds a whole group:
    arrival granularity = consumption granularity, and the 11 input
    DMAs fit the Tile scheduler's 8 DMA completion lanes without
    lane-slot stalls (measured: each extra dma_start costs ~600ns of
    sequencer issue time, and an over-subscribed lane blocks the ring).
  - First output half runs ko-OUTER (for ko: for t) with 8 live PSUM
    banks, consuming exactly one xw row per group in arrival order;
    second half runs t-outer, closing one tile per 1.7 us for a smooth
    bias-add + store drain.
  - The last tile is split 320+192 so the post-last-matmul chain is one
    short DVE add + one small store (the 320 piece stores on the other
    ring while the PE finishes the 192 piece).
  - Outputs are stored as fp16 (host upcasts), halving store traffic.
  - Eight wide (N=512) dummy matmuls bridge the PE from the preamble
    until the first input lands, so the HAM clock-ramp's ~3.4us busy
    window mostly elapses before the real stream starts. Wide dummies
    matter: narrow N=128 ones measured a net loss (~45-100ns each of
    queueing + per-instruction epilogue semaphore drain).
"""

import os

import numpy as np

import concourse.mybir as mybir
import concourse.tile as tile
from concourse import bacc
from concourse.bass_utils import run_bass_kernel_spmd

N_CORES = 8
B, IN_F, OUT_F = 8192, 1024, 1024
B_SHARD = B // N_CORES          # 1024 batch rows per core
P = 128                         # SBUF partitions
KO = IN_F // P                  # 8 contraction subtiles
NT = B_SHARD // P               # 8 batch tiles per core
NO = 2                          # 2 output column halves of 512
OW = OUT_F // NO                # 512 (one PSUM bank of fp32)

MODE = os.environ.get("BASS_KERNEL_MODE", "f16")
SPLIT_LAST = os.environ.get("BASS_KERNEL_SPLIT_LAST", "1") != "0"
# Merge per-tile output stores into a few multi-tile DMAs (out DRAM layout
# becomes [128, NT, 1024] r-major; host re-transposes). Fewer DMA
# instructions = less sequencer issue time + less NEFF epilogue drain.
MERGE_STORES = os.environ.get("BASS_KERNEL_MERGE_STORES", "0") != "0"
# Wide (N=512) PE warm-up dummies: few instructions (low epilogue tax) but
# still bridge the HAM clock-ramp window while the first input DMAs land.
N_WARM512 = int(os.environ.get("BASS_KERNEL_WARM512", "8"))

_nc_cache = {}


def _build(mode):
    assert mode == "f16", mode
    f32 = mybir.dt.float32
    f16 = mybir.dt.float16

    nc = bacc.Bacc("TRN2", target_bir_lowering=False)

    # DRAM layouts are host-packed so every DMA is contiguous per partition.
    # Each ko row of xw packs the W half-0 chunk AND the x slab one matmul
    # group consumes, so ONE DMA delivers a whole group's inputs:
    #   xw[ki, ko, 0:512]        = W[ko*128+ki, 0:512]      (half-0 chunk)
    #   xw[ki, ko, 512+t*128+bi] = x_shard[t*128+bi, ko*128+ki]
    #   w1[ki, ko, oi]           = W[512+oi, ko*128+ki]     (half 1)
    #   bias[p, o]               = b[o]  (host-replicated across partitions)
    XWW = OW + NT * P              # 1536 elements per ko row
    xw_d = nc.dram_tensor("xw", [P, KO, XWW], f16, kind="ExternalInput")
    w1_d = nc.dram_tensor("w1", [P, KO, OW], f16, kind="ExternalInput")
    bias_d = nc.dram_tensor("bias", [P, OUT_F], f32, kind="ExternalInput")
    if MERGE_STORES:
        # r-major: out[r, t, c] = result[t*128+r, c]; host re-transposes.
        out_d = nc.dram_tensor("out", [P, NT, OUT_F], f16, kind="ExternalOutput")
    else:
        out_d = nc.dram_tensor("out", [B_SHARD, OUT_F], f16, kind="ExternalOutput")

    with tile.TileContext(nc) as tc:
        with (
            tc.tile_pool(name="singles", bufs=1) as singles,
            tc.tile_pool(name="wpool", bufs=NO) as wpool,
            tc.tile_pool(name="xpool", bufs=1) as xpool,
            tc.tile_pool(name="opool", bufs=8) as opool,
            tc.tile_pool(name="psums", bufs=8, space="PSUM") as psums,
        ):
            bias_sb = singles.tile([P, OUT_F], f32)

            w1_sb = wpool.tile([P, KO, OW], f16, name="w1", tag="w_sb")
            xw_sb = xpool.tile([P, KO, XWW], f16, name="xw", tag="x_sb")

            # Optional PE warm-up: N_WARM512 wide (N=512) dummy matmuls on a
            # zeroed tile. Narrow N=128 dummies measured a net loss (~45-100ns
            # per instruction: queueing + epilogue semaphore drain); wide ones
            # bridge the same HAM clock-ramp window with 6x fewer
            # instructions. 0 disables (first ~8 real matmuls run cold).
            if N_WARM512:
                warm_sb = singles.tile([P, OW], f16)
                warm_ps = psums.tile([P, OW], f32, name="warm_ps", tag="ps")
                with tc.high_priority():
                    nc.gpsimd.memset(warm_sb[:], 0)
                    for _ in range(N_WARM512):
                        nc.tensor.matmul(
                            warm_ps[:], warm_sb[:, 0:P], warm_sb[:],
                            start=True, stop=True,
                        )

            # Input DMAs in strict consumption order: one packed xw DMA
            # feeds each 8-matmul group (arrival granularity = consumption
            # granularity), and the first 8 DMAs exactly fill the Tile
            # scheduler's 8 DMA completion lanes, so no issue ever blocks
            # on a lane slot mid-phase. The first group's row is split in
            # two halves across both rings so its W chunk + first x tiles
            # land in parallel as early as possible.
            nc.sync.dma_start(out=xw_sb[:, 0, 0:896], in_=xw_d[:, 0, 0:896])
            nc.scalar.dma_start(
                out=xw_sb[:, 0, 896:XWW], in_=xw_d[:, 0, 896:XWW]
            )
            for ko in range(1, KO):
                ring = nc.sync if ko % 2 else nc.scalar
                ring.dma_start(out=xw_sb[:, ko], in_=xw_d[:, ko])
            nc.scalar.dma_start(out=w1_sb[:, 0:4], in_=w1_d[:, 0:4])
            nc.sync.dma_start(out=w1_sb[:, 4:8], in_=w1_d[:, 4:8])
            nc.scalar.dma_start(out=bias_sb[:], in_=bias_d[:])

            # Output half 0: ko-outer over 8 live PSUM banks. Group ko needs
            # only x-slab ko + W chunk ko — one DMA each ahead of the PE.
            # The bias add + store for tile t are issued right after its
            # closing (stop) matmul so each gets its own PE semaphore tick
            # and drains while the remaining tiles still accumulate.
            ps0 = [
                psums.tile([P, OW], f32, name=f"ps0_{t}", tag="ps")
                for t in range(NT)
            ]
            if MERGE_STORES:
                o0_all = opool.tile([P, NT, OW], f16, name="o0_all", tag="o_sb")
                o1_all = opool.tile([P, 5, OW], f16, name="o1_all", tag="o_sb")
            for ko in range(KO):
                for t in range(NT):
                    nc.tensor.matmul(
                        ps0[t][:],
                        xw_sb[:, ko, OW + t * P:OW + (t + 1) * P],
                        xw_sb[:, ko, 0:OW],
                        start=(ko == 0),
                        stop=(ko == KO - 1),
                    )
                    if ko == KO - 1:
                        if MERGE_STORES:
                            nc.vector.tensor_add(
                                o0_all[:, t], ps0[t][:], bias_sb[:, 0:OW]
                            )
                            if t == NT - 1:
                                nc.sync.dma_start(
                                    out=out_d[:, :, 0:OW], in_=o0_all[:]
                                )
                        else:
                            o = opool.tile(
                                [P, OW], f16, name=f"o0_{t}", tag="o_sb"
                            )
                            nc.vector.tensor_add(
                                o[:], ps0[t][:], bias_sb[:, 0:OW]
                            )
                            nc.sync.dma_start(
                                out=out_d[t * P:(t + 1) * P, 0:OW], in_=o[:]
                            )

            # Output half 1: t-outer, one tile closes per group; the final
            # tile is split 320+192 to shorten the serial tail.
            for t in range(NT):
                if t < NT - 1 or not SPLIT_LAST:
                    ps = psums.tile([P, OW], f32, name=f"ps1_{t}", tag="ps")
                    for ko in range(KO):
                        nc.tensor.matmul(
                            ps[:],
                            xw_sb[:, ko, OW + t * P:OW + (t + 1) * P],
                            w1_sb[:, ko],
                            start=(ko == 0),
                            stop=(ko == KO - 1),
                        )
                    if MERGE_STORES and t < 5:
                        nc.vector.tensor_add(
                            o1_all[:, t], ps[:], bias_sb[:, OW:OUT_F]
                        )
                        if t == 4:
                            nc.sync.dma_start(
                                out=out_d[:, 0:5, OW:OUT_F], in_=o1_all[:]
                            )
                    else:
                        o = opool.tile([P, OW], f16, name=f"o1_{t}", tag="o_sb")
                        nc.vector.tensor_add(o[:], ps[:], bias_sb[:, OW:OUT_F])
                        dst = (out_d[:, t, OW:OUT_F] if MERGE_STORES
                               else out_d[t * P:(t + 1) * P, OW:OUT_F])
                        nc.sync.dma_start(out=dst, in_=o[:])
                else:
                    # Uneven 320+192 split: the big piece's add+store drain
                    # while the PE finishes the small one, so the
                    # post-last-matmul chain is one short DVE add + one
                    # store. The big piece stores via the (idle) ACT ring so
                    # the final store never queues behind it on SP. Sizes
                    # balance the DVE: the 320-add clears the engine just as
                    # the 192-piece's closing matmul lands.
                    for c0, c1 in ((OW, OW + 320), (OW + 320, OUT_F)):
                        cw = c1 - c0
                        ps = psums.tile(
                            [P, cw], f32, name=f"ps1_{t}_{c0}", tag="ps"
                        )
                        for ko in range(KO):
                            nc.tensor.matmul(
                                ps[:],
                                xw_sb[:, ko, OW + t * P:OW + (t + 1) * P],
                                w1_sb[:, ko, c0 - OW:c1 - OW],
                                start=(ko == 0),
                                stop=(ko == KO - 1),
                            )
                        o = opool.tile(
                            [P, cw], f16, name=f"o1_{t}_{c0}", tag="o_sb"
                        )
                        nc.vector.tensor_add(o[:], ps[:], bias_sb[:, c0:c1])
                        ring = nc.scalar if c1 != OUT_F else nc.sync
                        dst = (out_d[:, t, c0:c1] if MERGE_STORES
                               else out_d[t * P:(t + 1) * P, c0:c1])
                        ring.dma_start(out=dst, in_=o[:])
    nc.compile()
    return nc


def _get_nc(mode):
    if mode not in _nc_cache:
        _nc_cache[mode] = _build(mode)
    return _nc_cache[mode]


def _pack(x, W, b, mode="f16"):
    """Shard + retile host-side. Returns in_maps for the 8 cores."""
    x = np.asarray(x, dtype=np.float32)
    W = np.asarray(W, dtype=np.float32)
    b = np.asarray(b, dtype=np.float32)

    # [c, t, bi, ko, ki] -> [c, ki, ko, t*bi]
    xs = (
        x.reshape(N_CORES, NT, P, KO, P)
        .transpose(0, 4, 3, 1, 2)
        .reshape(N_CORES, P, KO, NT * P)
    )
    # [ot, oi, ko, ki] -> [ki, ot, ko, oi]
    ws = W.reshape(NO, OW, KO, P).transpose(3, 0, 2, 1)
    bias = np.ascontiguousarray(
        np.broadcast_to(b.reshape(1, OUT_F), (P, OUT_F))
    )

    xs16 = xs.astype(np.float16)
    ws16 = np.ascontiguousarray(ws).astype(np.float16)
    # xw[c, ki, ko, :] = [W half-0 chunk (512) | x slab (1024)]
    w0rep = np.broadcast_to(ws16[:, 0][None], (N_CORES, P, KO, OW))
    xw = np.ascontiguousarray(np.concatenate([w0rep, xs16], axis=-1))
    w1 = np.ascontiguousarray(ws16[:, 1])
    return [{"xw": xw[c], "w1": w1, "bias": bias} for c in range(N_CORES)]


def _run(in_maps, mode="f16", **kwargs):
    nc = _get_nc("f16")
    return run_bass_kernel_spmd(nc, in_maps, core_ids=list(range(N_CORES)), **kwargs)


def kernel(x, W, b):
    res = _run(_pack(x, W, b))
    outs = []
    for r in res.results:
        o = r["out"]
        if o.ndim == 3:  # MERGE_STORES r-major [128, NT, OUT_F]
            o = o.transpose(1, 0, 2).reshape(B_SHARD, OUT_F)
        outs.append(o)
    out = np.concatenate(outs, axis=0)
    return np.ascontiguousarray(out, dtype=np.float32)
